# revision 2
# baseline (speedup 1.0000x reference)
"""Top-1 MoE kernel for Trainium2 (8 NeuronCores, expert-parallel).

Problem shapes (hardcoded): B=4, T=2048, D=1024, H=4096, E=8.
reference returns (out, probs, expert_idx); out[b,t] = FFN_{e*}(x[b,t]) with
e* = argmax softmax(x @ Wr + br).

Strategy:
  - Host: router (logits/softmax/argmax in fp64), token dispatch: gather each
    expert's tokens into a padded [C, D] buffer (C = token capacity).
  - Device (SPMD, 1 expert per core): yT = W2.T @ relu(W1.T @ xT + b1) + b2
    computed entirely in transposed layout (features on partitions, tokens on
    the moving free dim), fp32r matmuls (full-rate TF32-like precision).
  - Host: scatter rows back, return (out, probs, expert_idx).

All weight/activation DRAM buffers are pre-tiled on the host into the exact
SBUF layouts so every device DMA is fully contiguous.
"""

import sys

import numpy as np

try:
    import concourse.bass as bass  # noqa: F401
except ImportError:  # pragma: no cover
    sys.path.insert(0, "/opt/trn_rl_repo")

import concourse.bacc as bacc
import concourse.mybir as mybir
import concourse.tile as tile
from concourse import bass_utils

P = 128
D = 1024
H = 4096
E = 8
KD = D // P   # 8  k-subtiles for layer-1 contraction
MH = H // P   # 32 h-tiles
MD = D // P   # 8  output d-tiles
NT = 384      # token chunk (moving free dim; >=256 keeps fp32r at full rate)
N_HALF = 2    # H split into halves so hT fits in SBUF
MH_HALF = MH // N_HALF  # 16

F32 = mybir.dt.float32
F32R = mybir.dt.float32r
AF = mybir.ActivationFunctionType

_COMPILED: dict[int, object] = {}


def _build(C: int):
    """Build + compile the per-core FFN program for token capacity C."""
    assert C % NT == 0
    n_chunks = C // NT
    chunks = [(i * NT, NT) for i in range(n_chunks)]

    nc = bacc.Bacc("TRN2", target_bir_lowering=False, debug=False)

    # Pre-tiled DRAM inputs (host prepares these exact layouts):
    #   xt[p, k, c]        = x_token[c, k*128+p]
    #   w1t[mh, p, k, m]   = W1[k*128+p, mh*128+m]
    #   w2t[half, dm, p, j, m] = W2[half*2048 + j*128 + p, dm*128+m]
    #   b1c[p, mh]         = b1[mh*128+p];  b2c[p, dm] = b2[dm*128+p]
    # Output: yt[p, dm, c] = y_token[c, dm*128+p]
    xt_d = nc.dram_tensor("xt", (P, KD, C), F32R, kind="ExternalInput").ap()
    w1_d = nc.dram_tensor("w1t", (MH, P, KD, P), F32R, kind="ExternalInput").ap()
    w2_d = nc.dram_tensor(
        "w2t", (N_HALF, MD, P, MH_HALF, P), F32R, kind="ExternalInput"
    ).ap()
    b1_d = nc.dram_tensor("b1c", (P, MH), F32, kind="ExternalInput").ap()
    b2_d = nc.dram_tensor("b2c", (P, MD), F32, kind="ExternalInput").ap()
    yt_d = nc.dram_tensor("yt", (P, MD, C), F32, kind="ExternalOutput").ap()

    with tile.TileContext(nc) as tc:
        with (
            tc.tile_pool(name="xp", bufs=1) as xp,
            tc.tile_pool(name="hp", bufs=1) as hp,
            tc.tile_pool(name="yp", bufs=1) as yp,
            tc.tile_pool(name="bp", bufs=1) as bp,
            tc.tile_pool(name="w1p", bufs=3) as w1p,
            tc.tile_pool(name="w2p", bufs=2) as w2p,
            tc.tile_pool(name="psp", bufs=6, space="PSUM") as psp,
        ):
            x_sb = xp.tile([P, KD, C], F32R)
            nc.sync.dma_start(x_sb[:], xt_d)
            b1_sb = bp.tile([P, MH], F32, tag="b1")
            b2_sb = bp.tile([P, MD], F32, tag="b2")
            nc.sync.dma_start(b1_sb[:], b1_d)
            nc.sync.dma_start(b2_sb[:], b2_d)
            y_sb = yp.tile([P, MD, C], F32)

            for half in range(N_HALF):
                # ---- layer 1 (this half of H): hT = relu(W1.T @ xT + b1)
                h_sb = hp.tile([P, MH_HALF, C], F32R)
                for m in range(MH_HALF):
                    mh = half * MH_HALF + m
                    w1t = w1p.tile([P, KD, P], F32R, tag="w1t")
                    nc.sync.dma_start(w1t[:], w1_d[mh])
                    for c0, nt in chunks:
                        pt = psp.tile([P, NT], F32, tag="pt")
                        for k in range(KD):
                            nc.tensor.matmul(
                                pt[:, :nt],
                                w1t[:, k],
                                x_sb[:, k, c0 : c0 + nt],
                                start=(k == 0),
                                stop=(k == KD - 1),
                            )
                        nc.scalar.activation(
                            h_sb[:, m, c0 : c0 + nt],
                            pt[:, :nt],
                            AF.Relu,
                            bias=b1_sb[:, mh : mh + 1],
                        )
                # ---- layer 2: yT += W2.T @ hT  (+ b2 on first half)
                for dm in range(MD):
                    w2t = w2p.tile([P, MH_HALF, P], F32R, tag="w2t")
                    nc.sync.dma_start(w2t[:], w2_d[half, dm])
                    for c0, nt in chunks:
                        pt = psp.tile([P, NT], F32, tag="pt")
                        for j in range(MH_HALF):
                            nc.tensor.matmul(
                                pt[:, :nt],
                                w2t[:, j],
                                h_sb[:, j, c0 : c0 + nt],
                                start=(j == 0),
                                stop=(j == MH_HALF - 1),
                            )
                        if half == 0:
                            nc.vector.tensor_scalar_add(
                                y_sb[:, dm, c0 : c0 + nt],
                                pt[:, :nt],
                                b2_sb[:, dm : dm + 1],
                            )
                        else:
                            nc.vector.tensor_add(
                                out=y_sb[:, dm, c0 : c0 + nt],
                                in0=y_sb[:, dm, c0 : c0 + nt],
                                in1=pt[:, :nt],
                            )
                            nc.sync.dma_start(
                                yt_d[:, dm, c0 : c0 + nt],
                                y_sb[:, dm, c0 : c0 + nt],
                            )

    nc.compile()
    return nc


def _get_kernel(C: int):
    if C not in _COMPILED:
        _COMPILED[C] = _build(C)
    return _COMPILED[C]


def kernel(x, Wr, br, W1, b1, W2, b2):
    B, T, _ = x.shape
    NTOK = B * T
    x2 = np.ascontiguousarray(x.reshape(NTOK, D), dtype=np.float32)

    # ---- host router (fp64 for a stable argmax; margins are ~2e-5 rel)
    logits = x2.astype(np.float64) @ Wr.astype(np.float64) + br.astype(np.float64)
    m = logits.max(axis=-1, keepdims=True)
    ex = np.exp(logits - m)
    probs = (ex / ex.sum(axis=-1, keepdims=True)).astype(np.float32)
    idx = np.argmax(logits, axis=-1).astype(np.int32)

    # ---- dispatch: group token ids by expert, pad to capacity C
    order = np.argsort(idx, kind="stable")
    counts = np.bincount(idx, minlength=E)
    C = max(1152, NT * int(np.ceil(counts.max() / NT)))
    nc = _get_kernel(C)

    starts = np.zeros(E + 1, dtype=np.int64)
    np.cumsum(counts, out=starts[1:])
    in_maps = []
    tok_ids = []
    for e in range(E):
        ids = order[starts[e] : starts[e + 1]]
        tok_ids.append(ids)
        xe = np.zeros((C, D), dtype=np.float32)
        xe[: counts[e]] = x2[ids]
        # tile to [P, KD, C]: xt[p,k,c] = xe[c, k*128+p]
        xt = np.ascontiguousarray(xe.T.reshape(KD, P, C).transpose(1, 0, 2))
        w1t = np.ascontiguousarray(
            W1[e].reshape(KD, P, MH, P).transpose(2, 1, 0, 3)
        )
        w2t = np.ascontiguousarray(
            W2[e].reshape(N_HALF, MH_HALF, P, MD, P).transpose(0, 3, 2, 1, 4)
        )
        b1c = np.ascontiguousarray(b1[e].reshape(MH, P).T)
        b2c = np.ascontiguousarray(b2[e].reshape(MD, P).T)
        in_maps.append(
            {"xt": xt, "w1t": w1t, "w2t": w2t, "b1c": b1c, "b2c": b2c}
        )

    res = bass_utils.run_bass_kernel_spmd(nc, in_maps, core_ids=list(range(E)))

    # ---- combine: un-tile yt [P, MD, C] -> [C, D], scatter into output
    out2 = np.empty((NTOK, D), dtype=np.float32)
    for e in range(E):
        yt = res.results[e]["yt"]  # [P, MD, C]
        ye = yt.transpose(1, 0, 2).reshape(D, C).T  # [C, D]
        out2[tok_ids[e]] = ye[: counts[e]]

    return out2.reshape(B, T, D), probs.reshape(B, T, E), idx.reshape(B, T)


# revision 5
# speedup vs baseline: 14.1260x; 14.1260x over previous
"""Top-1 MoE kernel for Trainium2 (8 NeuronCores, expert-parallel).

Problem shapes (hardcoded): B=4, T=2048, D=1024, H=4096, E=8.
reference returns (out, probs, expert_idx); out[b,t] = FFN_{e*}(x[b,t]) with
e* = argmax softmax(x @ Wr + br).

Strategy:
  - Host: router (logits/softmax/argmax in fp64), token dispatch: gather each
    expert's tokens into a padded [C, D] buffer (C = token capacity).
  - Device (SPMD, 1 expert per core): yT = W2.T @ relu(W1.T @ xT + b1) + b2
    computed entirely in transposed layout (features on partitions, tokens on
    the moving free dim), fp32r matmuls (full-rate TF32-like precision).
  - Host: scatter rows back, return (out, probs, expert_idx).

All weight/activation DRAM buffers are pre-tiled on the host into the exact
SBUF layouts so every device DMA is fully contiguous.
"""

import sys

import numpy as np

try:
    import concourse.bass as bass  # noqa: F401
except ImportError:  # pragma: no cover
    sys.path.insert(0, "/opt/trn_rl_repo")

import concourse.bacc as bacc
import concourse.mybir as mybir
import concourse.tile as tile
from concourse import bass_utils

P = 128
D = 1024
H = 4096
E = 8
KD = D // P   # 8  k-subtiles for layer-1 contraction
MH = H // P   # 32 h-tiles
MD = D // P   # 8  output d-tiles
NT = 384      # token chunk (moving free dim; >=256 keeps fp32r at full rate)
N_HALF = 2    # H split into halves so hT fits in SBUF
MH_HALF = MH // N_HALF  # 16

F32 = mybir.dt.float32
F32R = mybir.dt.float32r
AF = mybir.ActivationFunctionType

_COMPILED: dict[int, object] = {}


def emit_io_tensors(nc, C):
    """Pre-tiled DRAM I/O (host prepares these exact layouts):
      xt[p, k, c]            = x_token[c, k*128+p]
      w1t[mh, p, k, m]       = W1[k*128+p, mh*128+m]
      w2t[half, dm, p, j, m] = W2[half*2048 + j*128 + p, dm*128+m]
      b1c[p, mh]             = b1[mh*128+p];  b2c[p, dm] = b2[dm*128+p]
      yt[p, dm, c]           = y_token[c, dm*128+p]
    """
    return dict(
        xt=nc.dram_tensor("xt", (P, KD, C), F32R, kind="ExternalInput").ap(),
        w1t=nc.dram_tensor("w1t", (MH, P, KD, P), F32R, kind="ExternalInput").ap(),
        w2t=nc.dram_tensor(
            "w2t", (N_HALF, MD, P, MH_HALF, P), F32R, kind="ExternalInput"
        ).ap(),
        b1c=nc.dram_tensor("b1c", (P, MH), F32, kind="ExternalInput").ap(),
        b2c=nc.dram_tensor("b2c", (P, MD), F32, kind="ExternalInput").ap(),
        yt=nc.dram_tensor("yt", (P, MD, C), F32, kind="ExternalOutput").ap(),
    )


def emit_pools(tc, ctx):
    return dict(
        xp=ctx.enter_context(tc.tile_pool(name="xp", bufs=1)),
        hp=ctx.enter_context(tc.tile_pool(name="hp", bufs=1)),
        yp=ctx.enter_context(tc.tile_pool(name="yp", bufs=1)),
        bp=ctx.enter_context(tc.tile_pool(name="bp", bufs=1)),
        w1p=ctx.enter_context(tc.tile_pool(name="w1p", bufs=3)),
        w2p=ctx.enter_context(tc.tile_pool(name="w2p", bufs=2)),
        psp=ctx.enter_context(tc.tile_pool(name="psp", bufs=6, space="PSUM")),
    )


def emit_body(nc, io, pl, C, x_sb, y_sb, b1_sb, b2_sb, store_out=True):
    """One full FFN pass: y = relu(x@W1+b1)@W2 + b2, transposed layout.

    k-outer / chunk-inner matmul order: each loaded weight subtile feeds
    n_chunks consecutive matmuls so PE weight loads fully pipeline.
    """
    n_chunks = C // NT
    chunks = [(i * NT, NT) for i in range(n_chunks)]
    for half in range(N_HALF):
        # ---- layer 1 (this half of H): hT = relu(W1.T @ xT + b1)
        h_sb = pl["hp"].tile([P, MH_HALF, C], F32R)
        for m in range(MH_HALF):
            mh = half * MH_HALF + m
            w1t = pl["w1p"].tile([P, KD, P], F32R, tag="w1t")
            nc.sync.dma_start(w1t[:], io["w1t"][mh])
            pts = [
                pl["psp"].tile([P, NT], F32, tag="pt", name=f"pt_l1_{ci}")
                for ci in range(n_chunks)
            ]
            for k in range(KD):
                for ci, (c0, nt) in enumerate(chunks):
                    nc.tensor.matmul(
                        pts[ci][:, :nt],
                        w1t[:, k],
                        x_sb[:, k, c0 : c0 + nt],
                        start=(k == 0),
                        stop=(k == KD - 1),
                    )
            for ci, (c0, nt) in enumerate(chunks):
                nc.scalar.activation(
                    h_sb[:, m, c0 : c0 + nt],
                    pts[ci][:, :nt],
                    AF.Relu,
                    bias=b1_sb[:, mh : mh + 1],
                )
        # ---- layer 2: yT += W2.T @ hT  (+ b2 on first half)
        for dm in range(MD):
            w2t = pl["w2p"].tile([P, MH_HALF, P], F32R, tag="w2t")
            nc.sync.dma_start(w2t[:], io["w2t"][half, dm])
            pts = [
                pl["psp"].tile([P, NT], F32, tag="pt", name=f"pt_l2_{ci}")
                for ci in range(n_chunks)
            ]
            for j in range(MH_HALF):
                for ci, (c0, nt) in enumerate(chunks):
                    nc.tensor.matmul(
                        pts[ci][:, :nt],
                        w2t[:, j],
                        h_sb[:, j, c0 : c0 + nt],
                        start=(j == 0),
                        stop=(j == MH_HALF - 1),
                    )
            for ci, (c0, nt) in enumerate(chunks):
                if half == 0:
                    nc.vector.tensor_scalar_add(
                        y_sb[:, dm, c0 : c0 + nt],
                        pts[ci][:, :nt],
                        b2_sb[:, dm : dm + 1],
                    )
                else:
                    nc.vector.tensor_add(
                        out=y_sb[:, dm, c0 : c0 + nt],
                        in0=y_sb[:, dm, c0 : c0 + nt],
                        in1=pts[ci][:, :nt],
                    )
                    if store_out:
                        nc.sync.dma_start(
                            io["yt"][:, dm, c0 : c0 + nt],
                            y_sb[:, dm, c0 : c0 + nt],
                        )


def _build(C: int):
    """Build + compile the per-core FFN program for token capacity C."""
    assert C % NT == 0
    from contextlib import ExitStack

    nc = bacc.Bacc("TRN2", target_bir_lowering=False, debug=False)
    io = emit_io_tensors(nc, C)
    with tile.TileContext(nc) as tc, ExitStack() as ctx:
        pl = emit_pools(tc, ctx)
        x_sb = pl["xp"].tile([P, KD, C], F32R)
        for ci in range(C // NT):  # per-chunk loads so the first matmuls start early
            nc.sync.dma_start(
                x_sb[:, :, ci * NT : (ci + 1) * NT],
                io["xt"][:, :, ci * NT : (ci + 1) * NT],
            )
        b1_sb = pl["bp"].tile([P, MH], F32, tag="b1")
        b2_sb = pl["bp"].tile([P, MD], F32, tag="b2")
        nc.sync.dma_start(b1_sb[:], io["b1c"])
        nc.sync.dma_start(b2_sb[:], io["b2c"])
        y_sb = pl["yp"].tile([P, MD, C], F32)
        emit_body(nc, io, pl, C, x_sb, y_sb, b1_sb, b2_sb, store_out=True)
    nc.compile()
    return nc


def _get_kernel(C: int):
    if C not in _COMPILED:
        _COMPILED[C] = _build(C)
    return _COMPILED[C]


def host_prepare(x, Wr, br, W1, b1, W2, b2):
    """Router + dispatch on the host. Returns everything the device needs."""
    # accept jax or numpy inputs
    x, Wr, br, W1, b1, W2, b2 = (
        np.asarray(a) for a in (x, Wr, br, W1, b1, W2, b2)
    )
    B, T, _ = x.shape
    NTOK = B * T
    x2 = np.ascontiguousarray(x.reshape(NTOK, D), dtype=np.float32)

    # fp64 router for a stable argmax (top-2 margins are ~2e-5 relative)
    logits = x2.astype(np.float64) @ Wr.astype(np.float64) + br.astype(np.float64)
    m = logits.max(axis=-1, keepdims=True)
    ex = np.exp(logits - m)
    probs = (ex / ex.sum(axis=-1, keepdims=True)).astype(np.float32)
    idx = np.argmax(logits, axis=-1).astype(np.int32)

    order = np.argsort(idx, kind="stable")
    counts = np.bincount(idx, minlength=E)
    C = max(1152, NT * int(np.ceil(counts.max() / NT)))

    starts = np.zeros(E + 1, dtype=np.int64)
    np.cumsum(counts, out=starts[1:])
    in_maps, tok_ids = [], []
    for e in range(E):
        ids = order[starts[e] : starts[e + 1]]
        tok_ids.append(ids)
        xe = np.zeros((C, D), dtype=np.float32)
        xe[: counts[e]] = x2[ids]
        xt = np.ascontiguousarray(xe.T.reshape(KD, P, C).transpose(1, 0, 2))
        w1t = np.ascontiguousarray(W1[e].reshape(KD, P, MH, P).transpose(2, 1, 0, 3))
        w2t = np.ascontiguousarray(
            W2[e].reshape(N_HALF, MH_HALF, P, MD, P).transpose(0, 3, 2, 1, 4)
        )
        b1c = np.ascontiguousarray(b1[e].reshape(MH, P).T)
        b2c = np.ascontiguousarray(b2[e].reshape(MD, P).T)
        in_maps.append({"xt": xt, "w1t": w1t, "w2t": w2t, "b1c": b1c, "b2c": b2c})
    return x2, probs, idx, order, counts, C, in_maps, tok_ids


def kernel(x, Wr, br, W1, b1, W2, b2):
    B, T, _ = x.shape
    NTOK = B * T
    x2, probs, idx, order, counts, C, in_maps, tok_ids = host_prepare(
        x, Wr, br, W1, b1, W2, b2
    )
    nc = _get_kernel(C)
    res = bass_utils.run_bass_kernel_spmd(nc, in_maps, core_ids=list(range(E)))

    # ---- combine: un-tile yt [P, MD, C] -> [C, D], scatter into output
    out2 = np.empty((NTOK, D), dtype=np.float32)
    for e in range(E):
        yt = res.results[e]["yt"]  # [P, MD, C]
        ye = yt.transpose(1, 0, 2).reshape(D, C).T  # [C, D]
        out2[tok_ids[e]] = ye[: counts[e]]

    return out2.reshape(B, T, D), probs.reshape(B, T, E), idx.reshape(B, T)


# revision 7
# speedup vs baseline: 14.1788x; 1.0037x over previous
"""Top-1 MoE kernel for Trainium2 (8 NeuronCores, expert-parallel).

Problem shapes (hardcoded): B=4, T=2048, D=1024, H=4096, E=8.
reference returns (out, probs, expert_idx); out[b,t] = FFN_{e*}(x[b,t]) with
e* = argmax softmax(x @ Wr + br).

Strategy:
  - Host: router (logits/softmax/argmax in fp64), token dispatch: gather each
    expert's tokens into a padded [C, D] buffer (C = token capacity).
  - Device (SPMD, 1 expert per core): yT = W2.T @ relu(W1.T @ xT + b1) + b2
    computed entirely in transposed layout (features on partitions, tokens on
    the moving free dim), fp32r matmuls (full-rate TF32-like precision).
  - Host: scatter rows back, return (out, probs, expert_idx).

All weight/activation DRAM buffers are pre-tiled on the host into the exact
SBUF layouts so every device DMA is fully contiguous.
"""

import sys

import numpy as np

try:
    import concourse.bass as bass  # noqa: F401
except ImportError:  # pragma: no cover
    sys.path.insert(0, "/opt/trn_rl_repo")

import concourse.bacc as bacc
import concourse.mybir as mybir
import concourse.tile as tile
from concourse import bass_utils

P = 128
D = 1024
H = 4096
E = 8
KD = D // P   # 8  k-subtiles for layer-1 contraction
MH = H // P   # 32 h-tiles
MD = D // P   # 8  output d-tiles
NT = 364      # token chunk (moving free dim; >=256 keeps fp32r at full rate)
N_HALF = 2    # H split into halves so hT fits in SBUF
MH_HALF = MH // N_HALF  # 16

F32 = mybir.dt.float32
F32R = mybir.dt.float32r
AF = mybir.ActivationFunctionType

_COMPILED: dict[int, object] = {}


def emit_io_tensors(nc, C):
    """Pre-tiled DRAM I/O (host prepares these exact layouts):
      xt[p, k, c]            = x_token[c, k*128+p]
      w1t[mh, p, k, m]       = W1[k*128+p, mh*128+m]
      w2t[half, dm, p, j, m] = W2[half*2048 + j*128 + p, dm*128+m]
      b1c[p, mh]             = b1[mh*128+p];  b2c[p, dm] = b2[dm*128+p]
      yt[p, dm, c]           = y_token[c, dm*128+p]
    """
    return dict(
        xt=nc.dram_tensor("xt", (P, KD, C), F32R, kind="ExternalInput").ap(),
        w1t=nc.dram_tensor("w1t", (MH, P, KD, P), F32R, kind="ExternalInput").ap(),
        w2t=nc.dram_tensor(
            "w2t", (N_HALF, MD, P, MH_HALF, P), F32R, kind="ExternalInput"
        ).ap(),
        b1c=nc.dram_tensor("b1c", (P, MH), F32, kind="ExternalInput").ap(),
        b2c=nc.dram_tensor("b2c", (P, MD), F32, kind="ExternalInput").ap(),
        yt=nc.dram_tensor("yt", (P, MD, C), F32, kind="ExternalOutput").ap(),
    )


def emit_pools(tc, ctx):
    return dict(
        xp=ctx.enter_context(tc.tile_pool(name="xp", bufs=1)),
        hp=ctx.enter_context(tc.tile_pool(name="hp", bufs=1)),
        yp=ctx.enter_context(tc.tile_pool(name="yp", bufs=1)),
        bp=ctx.enter_context(tc.tile_pool(name="bp", bufs=1)),
        w1p=ctx.enter_context(tc.tile_pool(name="w1p", bufs=4)),
        w2p=ctx.enter_context(tc.tile_pool(name="w2p", bufs=2)),
        psp=ctx.enter_context(tc.tile_pool(name="psp", bufs=6, space="PSUM")),
    )


def emit_body(nc, io, pl, C, x_sb, y_sb, b1_sb, b2_sb, store_out=True):
    """One full FFN pass: y = relu(x@W1+b1)@W2 + b2, transposed layout.

    k-outer / chunk-inner matmul order: each loaded weight subtile feeds
    n_chunks consecutive matmuls so PE weight loads fully pipeline.
    """
    n_chunks = C // NT
    chunks = [(i * NT, NT) for i in range(n_chunks)]
    for half in range(N_HALF):
        # ---- layer 1 (this half of H): hT = relu(W1.T @ xT + b1)
        h_sb = pl["hp"].tile([P, MH_HALF, C], F32R)
        for m in range(MH_HALF):
            mh = half * MH_HALF + m
            w1t = pl["w1p"].tile([P, KD, P], F32R, tag="w1t")
            nc.sync.dma_start(w1t[:], io["w1t"][mh])
            pts = [
                pl["psp"].tile([P, NT], F32, tag="pt", name=f"pt_l1_{ci}")
                for ci in range(n_chunks)
            ]
            for k in range(KD):
                for ci, (c0, nt) in enumerate(chunks):
                    nc.tensor.matmul(
                        pts[ci][:, :nt],
                        w1t[:, k],
                        x_sb[:, k, c0 : c0 + nt],
                        start=(k == 0),
                        stop=(k == KD - 1),
                    )
            for ci, (c0, nt) in enumerate(chunks):
                nc.scalar.activation(
                    h_sb[:, m, c0 : c0 + nt],
                    pts[ci][:, :nt],
                    AF.Relu,
                    bias=b1_sb[:, mh : mh + 1],
                )
        # ---- layer 2: yT += W2.T @ hT  (+ b2 on first half)
        for dm in range(MD):
            w2t = pl["w2p"].tile([P, MH_HALF, P], F32R, tag="w2t")
            nc.sync.dma_start(w2t[:], io["w2t"][half, dm])
            pts = [
                pl["psp"].tile([P, NT], F32, tag="pt", name=f"pt_l2_{ci}")
                for ci in range(n_chunks)
            ]
            for j in range(MH_HALF):
                for ci, (c0, nt) in enumerate(chunks):
                    nc.tensor.matmul(
                        pts[ci][:, :nt],
                        w2t[:, j],
                        h_sb[:, j, c0 : c0 + nt],
                        start=(j == 0),
                        stop=(j == MH_HALF - 1),
                    )
            for ci, (c0, nt) in enumerate(chunks):
                if half == 0:
                    nc.vector.tensor_scalar_add(
                        y_sb[:, dm, c0 : c0 + nt],
                        pts[ci][:, :nt],
                        b2_sb[:, dm : dm + 1],
                    )
                else:
                    nc.vector.tensor_add(
                        out=y_sb[:, dm, c0 : c0 + nt],
                        in0=y_sb[:, dm, c0 : c0 + nt],
                        in1=pts[ci][:, :nt],
                    )
                    if store_out:
                        nc.sync.dma_start(
                            io["yt"][:, dm, c0 : c0 + nt],
                            y_sb[:, dm, c0 : c0 + nt],
                        )


def _build(C: int):
    """Build + compile the per-core FFN program for token capacity C."""
    assert C % NT == 0
    from contextlib import ExitStack

    nc = bacc.Bacc("TRN2", target_bir_lowering=False, debug=False)
    io = emit_io_tensors(nc, C)
    with tile.TileContext(nc) as tc, ExitStack() as ctx:
        pl = emit_pools(tc, ctx)
        x_sb = pl["xp"].tile([P, KD, C], F32R)
        for ci in range(C // NT):  # per-chunk loads so the first matmuls start early
            nc.sync.dma_start(
                x_sb[:, :, ci * NT : (ci + 1) * NT],
                io["xt"][:, :, ci * NT : (ci + 1) * NT],
            )
        b1_sb = pl["bp"].tile([P, MH], F32, tag="b1")
        b2_sb = pl["bp"].tile([P, MD], F32, tag="b2")
        nc.sync.dma_start(b1_sb[:], io["b1c"])
        nc.sync.dma_start(b2_sb[:], io["b2c"])
        y_sb = pl["yp"].tile([P, MD, C], F32)
        emit_body(nc, io, pl, C, x_sb, y_sb, b1_sb, b2_sb, store_out=True)
    nc.compile()
    return nc


def _get_kernel(C: int):
    if C not in _COMPILED:
        _COMPILED[C] = _build(C)
    return _COMPILED[C]


def host_prepare(x, Wr, br, W1, b1, W2, b2):
    """Router + dispatch on the host. Returns everything the device needs."""
    # accept jax or numpy inputs
    x, Wr, br, W1, b1, W2, b2 = (
        np.asarray(a) for a in (x, Wr, br, W1, b1, W2, b2)
    )
    B, T, _ = x.shape
    NTOK = B * T
    x2 = np.ascontiguousarray(x.reshape(NTOK, D), dtype=np.float32)

    # fp64 router for a stable argmax (top-2 margins are ~2e-5 relative)
    logits = x2.astype(np.float64) @ Wr.astype(np.float64) + br.astype(np.float64)
    m = logits.max(axis=-1, keepdims=True)
    ex = np.exp(logits - m)
    probs = (ex / ex.sum(axis=-1, keepdims=True)).astype(np.float32)
    idx = np.argmax(logits, axis=-1).astype(np.int32)

    order = np.argsort(idx, kind="stable")
    counts = np.bincount(idx, minlength=E)
    C = NT * max(3, int(np.ceil(counts.max() / NT)))

    starts = np.zeros(E + 1, dtype=np.int64)
    np.cumsum(counts, out=starts[1:])
    in_maps, tok_ids = [], []
    for e in range(E):
        ids = order[starts[e] : starts[e + 1]]
        tok_ids.append(ids)
        xe = np.zeros((C, D), dtype=np.float32)
        xe[: counts[e]] = x2[ids]
        xt = np.ascontiguousarray(xe.T.reshape(KD, P, C).transpose(1, 0, 2))
        w1t = np.ascontiguousarray(W1[e].reshape(KD, P, MH, P).transpose(2, 1, 0, 3))
        w2t = np.ascontiguousarray(
            W2[e].reshape(N_HALF, MH_HALF, P, MD, P).transpose(0, 3, 2, 1, 4)
        )
        b1c = np.ascontiguousarray(b1[e].reshape(MH, P).T)
        b2c = np.ascontiguousarray(b2[e].reshape(MD, P).T)
        in_maps.append({"xt": xt, "w1t": w1t, "w2t": w2t, "b1c": b1c, "b2c": b2c})
    return x2, probs, idx, order, counts, C, in_maps, tok_ids


def kernel(x, Wr, br, W1, b1, W2, b2):
    B, T, _ = x.shape
    NTOK = B * T
    x2, probs, idx, order, counts, C, in_maps, tok_ids = host_prepare(
        x, Wr, br, W1, b1, W2, b2
    )
    nc = _get_kernel(C)
    res = bass_utils.run_bass_kernel_spmd(nc, in_maps, core_ids=list(range(E)))

    # ---- combine: un-tile yt [P, MD, C] -> [C, D], scatter into output
    out2 = np.empty((NTOK, D), dtype=np.float32)
    for e in range(E):
        yt = res.results[e]["yt"]  # [P, MD, C]
        ye = yt.transpose(1, 0, 2).reshape(D, C).T  # [C, D]
        out2[tok_ids[e]] = ye[: counts[e]]

    return out2.reshape(B, T, D), probs.reshape(B, T, E), idx.reshape(B, T)


# revision 8
# speedup vs baseline: 15.1418x; 1.0679x over previous
"""Top-1 MoE kernel for Trainium2 (8 NeuronCores, expert-parallel).

Problem shapes (hardcoded): B=4, T=2048, D=1024, H=4096, E=8.
reference returns (out, probs, expert_idx); out[b,t] = FFN_{e*}(x[b,t]) with
e* = argmax softmax(x @ Wr + br).

Strategy:
  - Host: router (logits/softmax/argmax in fp64), token dispatch: gather each
    expert's tokens into a padded [C, D] buffer (C = token capacity).
  - Device (SPMD, 1 expert per core): yT = W2.T @ relu(W1.T @ xT + b1) + b2
    computed entirely in transposed layout (features on partitions, tokens on
    the moving free dim), fp32r matmuls (full-rate TF32-like precision).
  - Host: scatter rows back, return (out, probs, expert_idx).

All weight/activation DRAM buffers are pre-tiled on the host into the exact
SBUF layouts so every device DMA is fully contiguous.
"""

import sys

import numpy as np

try:
    import concourse.bass as bass  # noqa: F401
except ImportError:  # pragma: no cover
    sys.path.insert(0, "/opt/trn_rl_repo")

import concourse.bacc as bacc
import concourse.mybir as mybir
import concourse.tile as tile
from concourse import bass_utils

P = 128
D = 1024
H = 4096
E = 8
KD = D // P   # 8  k-subtiles for layer-1 contraction
MH = H // P   # 32 h-tiles
MD = D // P   # 8  output d-tiles
NT = 364      # token chunk (moving free dim; >=256 keeps fp32r at full rate)
N_HALF = 2    # H split into halves so hT fits in SBUF
MH_HALF = MH // N_HALF  # 16

F32 = mybir.dt.float32
F32R = mybir.dt.float32r
AF = mybir.ActivationFunctionType

_COMPILED: dict[int, object] = {}


def emit_io_tensors(nc, C):
    """Pre-tiled DRAM I/O (host prepares these exact layouts):
      xt[p, k, c]            = x_token[c, k*128+p]
      w1t[mh, p, k, m]       = W1[k*128+p, mh*128+m]
      w2t[half, dm, p, j, m] = W2[half*2048 + j*128 + p, dm*128+m]
      b1c[p, mh]             = b1[mh*128+p];  b2c[p, dm] = b2[dm*128+p]
      yt[p, dm, c]           = y_token[c, dm*128+p]
    """
    return dict(
        xt=nc.dram_tensor("xt", (P, KD, C), F32R, kind="ExternalInput").ap(),
        w1t=nc.dram_tensor("w1t", (MH, P, KD, P), F32R, kind="ExternalInput").ap(),
        w2t=nc.dram_tensor(
            "w2t", (N_HALF, MD, P, MH_HALF, P), F32R, kind="ExternalInput"
        ).ap(),
        b1c=nc.dram_tensor("b1c", (P, MH), F32, kind="ExternalInput").ap(),
        b2c=nc.dram_tensor("b2c", (P, MD), F32, kind="ExternalInput").ap(),
        yt=nc.dram_tensor("yt", (P, MD, C), F32, kind="ExternalOutput").ap(),
    )


def emit_pools(tc, ctx):
    return dict(
        xp=ctx.enter_context(tc.tile_pool(name="xp", bufs=1)),
        hp=ctx.enter_context(tc.tile_pool(name="hp", bufs=1)),
        yp=ctx.enter_context(tc.tile_pool(name="yp", bufs=1)),
        bp=ctx.enter_context(tc.tile_pool(name="bp", bufs=1)),
        w1p=ctx.enter_context(tc.tile_pool(name="w1p", bufs=4)),
        w2p=ctx.enter_context(tc.tile_pool(name="w2p", bufs=3)),
        psp=ctx.enter_context(tc.tile_pool(name="psp", bufs=6, space="PSUM")),
    )


def emit_body(nc, io, pl, C, x_sb, y_sb, b1_sb, b2_sb, store_out=True):
    """One full FFN pass: y = relu(x@W1+b1)@W2 + b2, transposed layout.

    k-outer / chunk-inner matmul order: each loaded weight subtile feeds
    n_chunks consecutive matmuls so PE weight loads fully pipeline.
    """
    n_chunks = C // NT
    chunks = [(i * NT, NT) for i in range(n_chunks)]
    for half in range(N_HALF):
        # ---- layer 1 (this half of H): hT = relu(W1.T @ xT + b1)
        h_sb = pl["hp"].tile([P, MH_HALF, C], F32R)
        for m in range(MH_HALF):
            mh = half * MH_HALF + m
            w1t = pl["w1p"].tile([P, KD, P], F32R, tag="w1t")
            nc.sync.dma_start(w1t[:], io["w1t"][mh])
            pts = [
                pl["psp"].tile([P, NT], F32, tag="pt", name=f"pt_l1_{ci}")
                for ci in range(n_chunks)
            ]
            for k in range(KD):
                for ci, (c0, nt) in enumerate(chunks):
                    nc.tensor.matmul(
                        pts[ci][:, :nt],
                        w1t[:, k],
                        x_sb[:, k, c0 : c0 + nt],
                        start=(k == 0),
                        stop=(k == KD - 1),
                    )
            for ci, (c0, nt) in enumerate(chunks):
                nc.scalar.activation(
                    h_sb[:, m, c0 : c0 + nt],
                    pts[ci][:, :nt],
                    AF.Relu,
                    bias=b1_sb[:, mh : mh + 1],
                )
        # ---- layer 2: yT += W2.T @ hT  (+ b2 on first half)
        for dm in range(MD):
            w2t = pl["w2p"].tile([P, MH_HALF, P], F32R, tag="w2t")
            nc.sync.dma_start(w2t[:], io["w2t"][half, dm])
            pts = [
                pl["psp"].tile([P, NT], F32, tag="pt", name=f"pt_l2_{ci}")
                for ci in range(n_chunks)
            ]
            for j in range(MH_HALF):
                for ci, (c0, nt) in enumerate(chunks):
                    nc.tensor.matmul(
                        pts[ci][:, :nt],
                        w2t[:, j],
                        h_sb[:, j, c0 : c0 + nt],
                        start=(j == 0),
                        stop=(j == MH_HALF - 1),
                    )
            for ci, (c0, nt) in enumerate(chunks):
                if half == 0:
                    nc.vector.tensor_scalar_add(
                        y_sb[:, dm, c0 : c0 + nt],
                        pts[ci][:, :nt],
                        b2_sb[:, dm : dm + 1],
                    )
                else:
                    nc.vector.tensor_add(
                        out=y_sb[:, dm, c0 : c0 + nt],
                        in0=y_sb[:, dm, c0 : c0 + nt],
                        in1=pts[ci][:, :nt],
                    )
                    if store_out:
                        nc.sync.dma_start(
                            io["yt"][:, dm, c0 : c0 + nt],
                            y_sb[:, dm, c0 : c0 + nt],
                        )


def _build(C: int):
    """Build + compile the per-core FFN program for token capacity C."""
    assert C % NT == 0
    from contextlib import ExitStack

    nc = bacc.Bacc("TRN2", target_bir_lowering=False, debug=False)
    io = emit_io_tensors(nc, C)
    with tile.TileContext(nc) as tc, ExitStack() as ctx:
        pl = emit_pools(tc, ctx)
        x_sb = pl["xp"].tile([P, KD, C], F32R)
        for ci in range(C // NT):  # per-chunk loads so the first matmuls start early
            nc.sync.dma_start(
                x_sb[:, :, ci * NT : (ci + 1) * NT],
                io["xt"][:, :, ci * NT : (ci + 1) * NT],
            )
        b1_sb = pl["bp"].tile([P, MH], F32, tag="b1")
        b2_sb = pl["bp"].tile([P, MD], F32, tag="b2")
        nc.sync.dma_start(b1_sb[:], io["b1c"])
        nc.sync.dma_start(b2_sb[:], io["b2c"])
        y_sb = pl["yp"].tile([P, MD, C], F32)
        emit_body(nc, io, pl, C, x_sb, y_sb, b1_sb, b2_sb, store_out=True)
    nc.compile()
    return nc


def _get_kernel(C: int):
    if C not in _COMPILED:
        _COMPILED[C] = _build(C)
    return _COMPILED[C]


def host_prepare(x, Wr, br, W1, b1, W2, b2):
    """Router + dispatch on the host. Returns everything the device needs."""
    # accept jax or numpy inputs
    x, Wr, br, W1, b1, W2, b2 = (
        np.asarray(a) for a in (x, Wr, br, W1, b1, W2, b2)
    )
    B, T, _ = x.shape
    NTOK = B * T
    x2 = np.ascontiguousarray(x.reshape(NTOK, D), dtype=np.float32)

    # fp64 router for a stable argmax (top-2 margins are ~2e-5 relative)
    logits = x2.astype(np.float64) @ Wr.astype(np.float64) + br.astype(np.float64)
    m = logits.max(axis=-1, keepdims=True)
    ex = np.exp(logits - m)
    probs = (ex / ex.sum(axis=-1, keepdims=True)).astype(np.float32)
    idx = np.argmax(logits, axis=-1).astype(np.int32)

    order = np.argsort(idx, kind="stable")
    counts = np.bincount(idx, minlength=E)
    C = NT * max(3, int(np.ceil(counts.max() / NT)))

    starts = np.zeros(E + 1, dtype=np.int64)
    np.cumsum(counts, out=starts[1:])
    in_maps, tok_ids = [], []
    for e in range(E):
        ids = order[starts[e] : starts[e + 1]]
        tok_ids.append(ids)
        xe = np.zeros((C, D), dtype=np.float32)
        xe[: counts[e]] = x2[ids]
        xt = np.ascontiguousarray(xe.T.reshape(KD, P, C).transpose(1, 0, 2))
        w1t = np.ascontiguousarray(W1[e].reshape(KD, P, MH, P).transpose(2, 1, 0, 3))
        w2t = np.ascontiguousarray(
            W2[e].reshape(N_HALF, MH_HALF, P, MD, P).transpose(0, 3, 2, 1, 4)
        )
        b1c = np.ascontiguousarray(b1[e].reshape(MH, P).T)
        b2c = np.ascontiguousarray(b2[e].reshape(MD, P).T)
        in_maps.append({"xt": xt, "w1t": w1t, "w2t": w2t, "b1c": b1c, "b2c": b2c})
    return x2, probs, idx, order, counts, C, in_maps, tok_ids


def kernel(x, Wr, br, W1, b1, W2, b2):
    B, T, _ = x.shape
    NTOK = B * T
    x2, probs, idx, order, counts, C, in_maps, tok_ids = host_prepare(
        x, Wr, br, W1, b1, W2, b2
    )
    nc = _get_kernel(C)
    res = bass_utils.run_bass_kernel_spmd(nc, in_maps, core_ids=list(range(E)))

    # ---- combine: un-tile yt [P, MD, C] -> [C, D], scatter into output
    out2 = np.empty((NTOK, D), dtype=np.float32)
    for e in range(E):
        yt = res.results[e]["yt"]  # [P, MD, C]
        ye = yt.transpose(1, 0, 2).reshape(D, C).T  # [C, D]
        out2[tok_ids[e]] = ye[: counts[e]]

    return out2.reshape(B, T, D), probs.reshape(B, T, E), idx.reshape(B, T)
